# revision 28
# baseline (speedup 1.0000x reference)
"""Trainium2 Bass kernel: fused multi-head self-attention + output projection.

Problem (fixed shapes):
    N=2, S=2048, EMBED=1024, HEADS=16, HEAD_DIM=64, mask == all-ones.
    energy = einsum('nqhd,nkhd->nhqk', Q, K)
    attn   = softmax(energy / sqrt(EMBED), axis=k)
    out    = einsum('nhqk,nkhd->nqhd', attn, V).reshape(N,S,E) @ W_out.T + b_out

Sharding across 8 NeuronCores: core i handles batch n = i//4 and the 4 heads
[4g, 4g+4) with g = i%4 (data parallel over batch, tensor parallel over
heads).  Each core computes attention for its 4 heads plus the partial output
projection against the matching 256-row slice of W_out.T; the host sums the 4
partials per batch and adds b_out.

Device-side layout (everything stays transposed; no on-chip transposes, all
matmul operands bf16 — the only full-rate PE dtype; fp32r lowers to the
half-speed FP32-HIGH path):
    energyT[ki,qi] = matmul(lhsT=kT, rhs=qT)        (2 heads row-packed, ->PSUM f32)
    PT = exp(energyT/32)                            (ScalarE, 1024-wide, ->bf16)
    aoT[65,qi]    += matmul(lhsT=[v|1], rhs=PT)     (65th row = softmax denom)
    aonT = aoT[0:64] * bcast(1/aoT[64])             (DVE recip + GpSimd broadcast)
    proj[qi,e]    += matmul(lhsT=aonT, rhs=W'_h)    (accumulated over 4 heads)

The producer/consumer groups are software-pipelined (energy+exp of group g
emitted alongside the AV matmuls of group g-1, projection work drip-fed one
job per kc tick) so ScalarE — the 143us/core exp floor — never starves and
TensorE gaps stay under the ~3.4us HAM re-throttle window.
"""

import numpy as np

N, S, E, H, D = 2, 2048, 1024, 16, 64
P = 128                 # SBUF/PSUM partitions
QB = 512                # qi block width (PE moving-operand max for fp32)
KC = S // P             # 16 ki chunks of 128
NB = S // QB            # 4 qi blocks
HPC = 4                 # heads per core
SCALE = 1.0 / 32.0      # 1/sqrt(EMBED)

_PROGRAM = None


def _build_program():
    import concourse.bacc as bacc
    import concourse.mybir as mybir
    import concourse.tile as tile

    f32 = mybir.dt.float32
    bf16 = mybir.dt.bfloat16
    Exp = mybir.ActivationFunctionType.Exp

    nc = bacc.Bacc("TRN2", target_bir_lowering=False)

    qt_d = nc.dram_tensor("qt", [2, P, S], bf16, kind="ExternalInput")
    kt_d = nc.dram_tensor("kt", [2, P, S], bf16, kind="ExternalInput")
    v_d = nc.dram_tensor("v", [HPC, S, D], bf16, kind="ExternalInput")
    wt_d = nc.dram_tensor("wt", [2, P, E], bf16, kind="ExternalInput")
    out_d = nc.dram_tensor("out", [S, E], f32, kind="ExternalOutput")

    with tile.TileContext(nc) as tc:
        from contextlib import ExitStack

        with ExitStack() as ctx:
            singles = ctx.enter_context(tc.tile_pool(name="singles", bufs=1))
            ptp = ctx.enter_context(tc.tile_pool(name="ptp", bufs=36))
            rcp = ctx.enter_context(tc.tile_pool(name="rcp", bufs=4))
            bcp = ctx.enter_context(tc.tile_pool(name="bcp", bufs=3))
            tmpp = ctx.enter_context(tc.tile_pool(name="tmpp", bufs=2))
            outp = ctx.enter_context(tc.tile_pool(name="outp", bufs=4))
            epp = ctx.enter_context(tc.tile_pool(name="epp", bufs=2, space="PSUM"))
            aop = ctx.enter_context(tc.tile_pool(name="aop", bufs=2, space="PSUM"))
            ppp = ctx.enter_context(tc.tile_pool(name="ppp", bufs=2, space="PSUM"))

            # ---- persistent inputs -------------------------------------------------
            # one SBUF tensor per head for q/k, with head hh of pair p parked at
            # partitions [64*hh, 64*hh+64) (row-packed matmul pairs then stream
            # from distinct tensors, giving the XBUSes independent sources)
            qh = [singles.tile([P, S], bf16, tag=f"qh{i}", name=f"qh{i}") for i in range(4)]
            kh = [singles.tile([P, S], bf16, tag=f"kh{i}", name=f"kh{i}") for i in range(4)]
            # loads ordered by first use: group 0 consumes all of kh0/kh1 (ki
            # axis) but only the first qi block of qh0/qh1; v is needed by the
            # first AV matmuls (~20us in); later qi blocks of q come last.
            def load_qk(i, cc, eng):
                p, hh = divmod(i, 2)
                cs = slice(cc * QB, (cc + 1) * QB)
                sl = slice(hh * D, (hh + 1) * D)
                eng.dma_start(out=kh[i][sl, cs] if eng is nc.sync else qh[i][sl, cs],
                              in_=(kt_d if eng is nc.sync else qt_d)[p, sl, cs])
            # v per head: [128, kc, 65] bf16, 65th column = 1.0 (denominator trick)
            # [v | 1] per head: column 64 = ones => aoT row 64 = softmax denom
            vt = [singles.tile([P, KC, D + 1], bf16, tag=f"vt{h}", name=f"vt{h}") for h in range(HPC)]
            wt = [singles.tile([P, E], bf16, tag=f"wt{h}", name=f"wt{h}") for h in range(2)]
            for cc in range(4):
                for i in range(2):
                    load_qk(i, cc, nc.sync)      # kh0/kh1, all ki chunks
            for i in range(2):
                load_qk(i, 0, nc.gpsimd)         # qh0/qh1 first qi block
            for h in range(HPC):
                nc.gpsimd.dma_start(
                    out=vt[h][:, :, 0:D],
                    in_=v_d[h].rearrange("(c p) d -> p c d", p=P),
                )
                nc.vector.memset(vt[h][:, :, D : D + 1], 1.0)
            for cc in range(4):
                for i in range(2, 4):
                    load_qk(i, cc, nc.sync)      # kh2/kh3
            for cc in range(1, 4):
                for i in range(2):
                    load_qk(i, cc, nc.gpsimd)    # qh0/qh1 remaining qi blocks
            for cc in range(4):
                for i in range(2, 4):
                    load_qk(i, cc, nc.gpsimd)    # qh2/qh3
            for h in range(2):
                nc.sync.dma_start(out=wt[h], in_=wt_d[h])
            # normalized attention outputs, transposed: [128, S] per head PAIR
            # (odd head occupies partitions 64-127 via a partition-shifting
            # SBUF->SBUF DMA, enabling full-depth contract-128 projection)
            aont = [singles.tile([P, S], bf16, tag=f"aont{pr}", name=f"aont{pr}") for pr in range(2)]
            # dummy exp: pulls the ACT table load into the DMA-wait window
            warm = singles.tile([1, 1], f32, tag="warm", name="warm")
            nc.vector.memset(warm, 0.0)
            nc.scalar.activation(warm, warm, Exp, scale=1.0)

            # ---- software-pipelined main loop --------------------------------------
            # groups: (qi block B, head pair p); produce (energy+exp) for group gi
            # while consuming (AV matmuls) group gi-1 so ScalarE never starves.
            groups = [(B, p) for B in range(NB) for p in range(2)]
            pts = {}  # gi -> list of 16 PT tiles
            proj_jobs = []  # deferred projection thunks, drip-fed into kc loops
            proj_cooldown = [0]  # ticks to wait before dripping fresh jobs

            def emit_proj(Bc):
                for j in range(Bc * 4, Bc * 4 + 4):
                    ob = outp.tile([P, E], f32, tag="ob", name="ob")
                    for eb in range(2):

                        def mm_job(j=j, eb=eb, ob=ob):
                            pp = ppp.tile([P, QB], f32, tag="pp", name="pp")
                            for pr in range(2):
                                nc.tensor.matmul(
                                    pp,
                                    lhsT=aont[pr][:, j * P : (j + 1) * P],
                                    rhs=wt[pr][:, eb * QB : (eb + 1) * QB],
                                    start=(pr == 0),
                                    stop=(pr == 1),
                                )
                            nc.vector.tensor_copy(ob[:, eb * QB : (eb + 1) * QB], pp)
                            nc.sync.dma_start(
                                out=out_d[j * P : (j + 1) * P, eb * QB : (eb + 1) * QB],
                                in_=ob[:, eb * QB : (eb + 1) * QB],
                            )

                        proj_jobs.append(mm_job)

            def normalize(cons, ao, hh):
                Bc, pc = cons
                # stage the denom row to SBUF partition 0: custom-DVE ops
                # only address base partition 0 correctly on HW, and engine
                # APs must start 32-aligned.
                rc0 = rcp.tile([1, QB], f32, tag="rc0", name="rc0")
                nc.vector.tensor_copy(rc0, ao[hh][D : D + 1, :])
                rc = rcp.tile([1, QB], f32, tag="rc", name="rc")
                nc.vector.reciprocal_approx_fast(out=rc, in_=rc0)
                bc = bcp.tile([D, QB], f32, tag="bc", name="bc")
                nc.gpsimd.partition_broadcast(bc, rc, channels=D)
                if hh == 0:
                    nc.vector.tensor_mul(
                        aont[pc][0:D, Bc * QB : (Bc + 1) * QB], ao[hh][0:D, :], bc
                    )
                else:
                    # engine writes can't start at partition 64 from a base-0
                    # source; stage and partition-shift via DMA
                    tmp = tmpp.tile([D, QB], bf16, tag="tmp", name="tmp")
                    nc.vector.tensor_mul(tmp, ao[hh][0:D, :], bc)
                    nc.sync.dma_start(
                        out=aont[pc][D:P, Bc * QB : (Bc + 1) * QB], in_=tmp
                    )

            for gi in range(len(groups) + 1):
                prod = groups[gi] if gi < len(groups) else None
                cons = groups[gi - 1] if gi >= 1 else None
                if prod is not None:
                    pts[gi] = []
                if cons is not None:
                    ao = [aop.tile([D + 1, QB], f32, tag="ao", name="ao") for _ in range(2)]
                for kc in range(KC):
                    if proj_cooldown[0] > 0:
                        proj_cooldown[0] -= 1
                    else:
                        for _ in range(2 if prod is None else 1):
                            if proj_jobs:
                                proj_jobs.pop(0)()
                    if prod is not None:
                        B, p = prod
                        e = epp.tile([P, 2 * QB], f32, tag="ep", name="ep")
                        for hh in range(2):
                            i = 2 * p + hh
                            sl = slice(hh * D, (hh + 1) * D)
                            nc.tensor.matmul(
                                e[:, hh * QB : (hh + 1) * QB],
                                lhsT=kh[i][sl, kc * P : (kc + 1) * P],
                                rhs=qh[i][sl, B * QB : (B + 1) * QB],
                                start=True,
                                stop=True,
                            )
                        t = ptp.tile([P, 2 * QB], bf16, tag="pt", name="pt")
                        nc.scalar.activation(t, e, Exp, scale=SCALE)
                        pts[gi].append(t)
                    if cons is not None:
                        # h-major: h0's 16 AV matmuls over ticks 0-7, h1 over
                        # 8-15.  h1's ao-slot wait (previous group's h0
                        # normalize) hides behind h0's work, and each head's
                        # normalize chain starts half a group earlier.
                        Bc, pc = cons
                        hh = 0 if kc < KC // 2 else 1
                        for q in range(2):
                            k2 = (kc % (KC // 2)) * 2 + q
                            nc.tensor.matmul(
                                ao[hh],
                                lhsT=vt[2 * pc + hh][:, k2, :],
                                rhs=pts[gi - 1][k2][:, hh * QB : (hh + 1) * QB],
                                start=(k2 == 0),
                                stop=(k2 == KC - 1),
                            )
                        if kc == KC // 2 - 1:
                            normalize(cons, ao, 0)
                if cons is not None:
                    Bc, pc = cons
                    normalize(cons, ao, 1)
                    del pts[gi - 1]
                    if pc == 1:
                        # all 4 heads of qi block Bc are normalized: queue its
                        # projection, drip-fed into upcoming kc loops so it
                        # never blocks energy production (ScalarE supply).
                        # cooldown: don't pop the first job until the aont
                        # writes have had time to land (in-order PE queue).
                        if Bc < NB - 1:
                            emit_proj(Bc)
                            proj_cooldown[0] = 4
            # final-block flush: pr0 accumulations start immediately (their
            # aont half was normalized a full group earlier); pr1 + copy
            # retire in straddled pairs across the 2 pp slots.
            flush = [(j, eb) for j in range((NB - 1) * 4, NB * 4) for eb in range(2)]
            obs = {}
            for j in range((NB - 1) * 4, NB * 4):
                obs[j] = outp.tile([P, E], f32, tag="ob", name="ob")
            pend = []
            for idx, (j, eb) in enumerate(flush):
                pp = ppp.tile([P, QB], f32, tag="pp", name="pp")
                nc.tensor.matmul(
                    pp,
                    lhsT=aont[0][:, j * P : (j + 1) * P],
                    rhs=wt[0][:, eb * QB : (eb + 1) * QB],
                    start=True,
                    stop=False,
                )
                pend.append((j, eb, pp))
                if len(pend) == 2 or idx == len(flush) - 1:
                    for j2, eb2, pp2 in pend:
                        nc.tensor.matmul(
                            pp2,
                            lhsT=aont[1][:, j2 * P : (j2 + 1) * P],
                            rhs=wt[1][:, eb2 * QB : (eb2 + 1) * QB],
                            start=False,
                            stop=True,
                        )
                        nc.vector.tensor_copy(obs[j2][:, eb2 * QB : (eb2 + 1) * QB], pp2)
                        nc.sync.dma_start(
                            out=out_d[j2 * P : (j2 + 1) * P, eb2 * QB : (eb2 + 1) * QB],
                            in_=obs[j2][:, eb2 * QB : (eb2 + 1) * QB],
                        )
                    pend = []

    nc.compile()
    return nc




def _program():
    global _PROGRAM
    if _PROGRAM is None:
        _PROGRAM = _build_program()
    return _PROGRAM


def _shard_inputs(values, keys, query, W_out):
    import ml_dtypes

    q = np.ascontiguousarray(np.asarray(query, np.float32)).reshape(N, S, H, D)
    k = np.ascontiguousarray(np.asarray(keys, np.float32)).reshape(N, S, H, D)
    v = np.ascontiguousarray(np.asarray(values, np.float32)).reshape(N, S, H, D)
    qT = np.ascontiguousarray(q.transpose(0, 2, 3, 1))  # [N, H, D, S]
    kT = np.ascontiguousarray(k.transpose(0, 2, 3, 1))
    vh = v.transpose(0, 2, 1, 3)  # [N, H, S, D] (view)
    WT = np.ascontiguousarray(np.asarray(W_out, np.float32).T)  # [E_in, E_out]

    in_maps = []
    for i in range(8):
        n, g = i // 4, i % 4
        h0 = 4 * g
        in_maps.append(
            {
                "qt": np.ascontiguousarray(qT[n, h0 : h0 + 4]).reshape(2, P, S).astype(ml_dtypes.bfloat16),
                "kt": np.ascontiguousarray(kT[n, h0 : h0 + 4]).reshape(2, P, S).astype(ml_dtypes.bfloat16),
                "v": np.ascontiguousarray(vh[n, h0 : h0 + 4]).astype(ml_dtypes.bfloat16),
                "wt": np.ascontiguousarray(WT[256 * g : 256 * (g + 1)]).reshape(2, P, E).astype(ml_dtypes.bfloat16),
            }
        )
    return in_maps


def kernel(values, keys, query, mask, W_out, b_out, _trace=False, _bkr_out=None):
    """Full inputs in, full output out.  mask is all-ones by construction and
    is ignored.  _trace/_bkr_out are test hooks (NTFF profiling)."""
    from concourse.bass_utils import run_bass_kernel_spmd

    nc = _program()
    in_maps = _shard_inputs(values, keys, query, W_out)
    bkr = run_bass_kernel_spmd(nc, in_maps, list(range(8)), trace=_trace)
    if _bkr_out is not None:
        _bkr_out.append(bkr)

    b = np.asarray(b_out, np.float32)
    out = np.empty((N, S, E), np.float32)
    for n in range(2):
        acc = bkr.results[4 * n]["out"].astype(np.float64)
        for j in range(1, 4):
            acc += bkr.results[4 * n + j]["out"]
        out[n] = (acc + b).astype(np.float32)
    return out



# revision 29
# speedup vs baseline: 1.0140x; 1.0140x over previous
"""Trainium2 Bass kernel: fused multi-head self-attention + output projection.

Problem (fixed shapes):
    N=2, S=2048, EMBED=1024, HEADS=16, HEAD_DIM=64, mask == all-ones.
    energy = einsum('nqhd,nkhd->nhqk', Q, K)
    attn   = softmax(energy / sqrt(EMBED), axis=k)
    out    = einsum('nhqk,nkhd->nqhd', attn, V).reshape(N,S,E) @ W_out.T + b_out

Sharding across 8 NeuronCores: core i handles batch n = i//4 and the 4 heads
[4g, 4g+4) with g = i%4 (data parallel over batch, tensor parallel over
heads).  Each core computes attention for its 4 heads plus the partial output
projection against the matching 256-row slice of W_out.T; the host sums the 4
partials per batch and adds b_out.

Device-side layout (everything stays transposed; no on-chip transposes, all
matmul operands bf16 — the only full-rate PE dtype; fp32r lowers to the
half-speed FP32-HIGH path):
    energyT[ki,qi] = matmul(lhsT=kT, rhs=qT)        (2 heads row-packed, ->PSUM f32)
    PT = exp(energyT/32)                            (ScalarE, 1024-wide, ->bf16)
    aoT[65,qi]    += matmul(lhsT=[v|1], rhs=PT)     (65th row = softmax denom)
    aonT = aoT[0:64] * bcast(1/aoT[64])             (DVE recip + GpSimd broadcast)
    proj[qi,e]    += matmul(lhsT=aonT, rhs=W'_h)    (accumulated over 4 heads)

The producer/consumer groups are software-pipelined (energy+exp of group g
emitted alongside the AV matmuls of group g-1, projection work drip-fed one
job per kc tick) so ScalarE — the 143us/core exp floor — never starves and
TensorE gaps stay under the ~3.4us HAM re-throttle window.
"""

import numpy as np

N, S, E, H, D = 2, 2048, 1024, 16, 64
P = 128                 # SBUF/PSUM partitions
QB = 512                # qi block width (PE moving-operand max for fp32)
KC = S // P             # 16 ki chunks of 128
NB = S // QB            # 4 qi blocks
HPC = 4                 # heads per core
SCALE = 1.0 / 32.0      # 1/sqrt(EMBED)

_PROGRAM = None


def _build_program():
    import concourse.bacc as bacc
    import concourse.mybir as mybir
    import concourse.tile as tile

    f32 = mybir.dt.float32
    bf16 = mybir.dt.bfloat16
    Exp = mybir.ActivationFunctionType.Exp

    nc = bacc.Bacc("TRN2", target_bir_lowering=False)

    qt_d = nc.dram_tensor("qt", [2, P, S], bf16, kind="ExternalInput")
    kt_d = nc.dram_tensor("kt", [2, P, S], bf16, kind="ExternalInput")
    v_d = nc.dram_tensor("v", [HPC, S, D], bf16, kind="ExternalInput")
    wt_d = nc.dram_tensor("wt", [2, P, E], bf16, kind="ExternalInput")
    out_d = nc.dram_tensor("out", [S, E], f32, kind="ExternalOutput")

    with tile.TileContext(nc) as tc:
        from contextlib import ExitStack

        with ExitStack() as ctx:
            singles = ctx.enter_context(tc.tile_pool(name="singles", bufs=1))
            ptp = ctx.enter_context(tc.tile_pool(name="ptp", bufs=36))
            rcp = ctx.enter_context(tc.tile_pool(name="rcp", bufs=4))
            bcp = ctx.enter_context(tc.tile_pool(name="bcp", bufs=3))
            tmpp = ctx.enter_context(tc.tile_pool(name="tmpp", bufs=2))
            outp = ctx.enter_context(tc.tile_pool(name="outp", bufs=3))
            epp = ctx.enter_context(tc.tile_pool(name="epp", bufs=2, space="PSUM"))
            aop = ctx.enter_context(tc.tile_pool(name="aop", bufs=2, space="PSUM"))
            ppp = ctx.enter_context(tc.tile_pool(name="ppp", bufs=2, space="PSUM"))

            # ---- persistent inputs -------------------------------------------------
            # one SBUF tensor per head for q/k, with head hh of pair p parked at
            # partitions [64*hh, 64*hh+64) (row-packed matmul pairs then stream
            # from distinct tensors, giving the XBUSes independent sources)
            qh = [singles.tile([P, S], bf16, tag=f"qh{i}", name=f"qh{i}") for i in range(4)]
            kh = [singles.tile([P, S], bf16, tag=f"kh{i}", name=f"kh{i}") for i in range(4)]
            # loads ordered by first use: group 0 consumes all of kh0/kh1 (ki
            # axis) but only the first qi block of qh0/qh1; v is needed by the
            # first AV matmuls (~20us in); later qi blocks of q come last.
            def load_qk(i, cc, eng):
                p, hh = divmod(i, 2)
                cs = slice(cc * QB, (cc + 1) * QB)
                sl = slice(hh * D, (hh + 1) * D)
                eng.dma_start(out=kh[i][sl, cs] if eng is nc.sync else qh[i][sl, cs],
                              in_=(kt_d if eng is nc.sync else qt_d)[p, sl, cs])
            # v per head: [128, kc, 65] bf16, 65th column = 1.0 (denominator trick)
            # [v | 1] per head: column 64 = ones => aoT row 64 = softmax denom
            vt = [singles.tile([P, KC, D + 1], bf16, tag=f"vt{h}", name=f"vt{h}") for h in range(HPC)]
            wt = [singles.tile([P, E], bf16, tag=f"wt{h}", name=f"wt{h}") for h in range(2)]
            for cc in range(4):
                for i in range(2):
                    load_qk(i, cc, nc.sync)      # kh0/kh1, all ki chunks
            for i in range(2):
                load_qk(i, 0, nc.gpsimd)         # qh0/qh1 first qi block
            for h in range(HPC):
                nc.gpsimd.dma_start(
                    out=vt[h][:, :, 0:D],
                    in_=v_d[h].rearrange("(c p) d -> p c d", p=P),
                )
                nc.vector.memset(vt[h][:, :, D : D + 1], 1.0)
            for cc in range(4):
                for i in range(2, 4):
                    load_qk(i, cc, nc.sync)      # kh2/kh3
            for cc in range(1, 4):
                for i in range(2):
                    load_qk(i, cc, nc.gpsimd)    # qh0/qh1 remaining qi blocks
            for cc in range(4):
                for i in range(2, 4):
                    load_qk(i, cc, nc.gpsimd)    # qh2/qh3
            for h in range(2):
                nc.sync.dma_start(out=wt[h], in_=wt_d[h])
            # normalized attention outputs, transposed: [128, S] per head PAIR
            # (odd head occupies partitions 64-127 via a partition-shifting
            # SBUF->SBUF DMA, enabling full-depth contract-128 projection)
            aont = [singles.tile([P, S], bf16, tag=f"aont{pr}", name=f"aont{pr}") for pr in range(2)]
            # dummy exp: pulls the ACT table load into the DMA-wait window
            warm = singles.tile([1, 1], f32, tag="warm", name="warm")
            nc.vector.memset(warm, 0.0)
            nc.scalar.activation(warm, warm, Exp, scale=1.0)

            # ---- software-pipelined main loop --------------------------------------
            # groups: (qi block B, head pair p); produce (energy+exp) for group gi
            # while consuming (AV matmuls) group gi-1 so ScalarE never starves.
            groups = [(B, p) for B in range(NB) for p in range(2)]
            pts = {}  # gi -> list of 16 PT tiles
            proj_jobs = []  # deferred projection thunks, drip-fed into kc loops
            proj_cooldown = [0]  # ticks to wait before dripping fresh jobs

            def emit_proj(Bc):
                for j in range(Bc * 4, Bc * 4 + 4):
                    ob = outp.tile([P, E], f32, tag="ob", name="ob")
                    for eb in range(2):

                        def mm_job(j=j, eb=eb, ob=ob):
                            pp = ppp.tile([P, QB], f32, tag="pp", name="pp")
                            for pr in range(2):
                                nc.tensor.matmul(
                                    pp,
                                    lhsT=aont[pr][:, j * P : (j + 1) * P],
                                    rhs=wt[pr][:, eb * QB : (eb + 1) * QB],
                                    start=(pr == 0),
                                    stop=(pr == 1),
                                )
                            nc.vector.tensor_copy(ob[:, eb * QB : (eb + 1) * QB], pp)
                            nc.sync.dma_start(
                                out=out_d[j * P : (j + 1) * P, eb * QB : (eb + 1) * QB],
                                in_=ob[:, eb * QB : (eb + 1) * QB],
                            )

                        proj_jobs.append(mm_job)

            def normalize(cons, ao, hh):
                Bc, pc = cons
                # stage the denom row to SBUF partition 0: custom-DVE ops
                # only address base partition 0 correctly on HW, and engine
                # APs must start 32-aligned.
                rc0 = rcp.tile([1, QB], f32, tag="rc0", name="rc0")
                nc.vector.tensor_copy(rc0, ao[hh][D : D + 1, :])
                rc = rcp.tile([1, QB], f32, tag="rc", name="rc")
                nc.vector.reciprocal_approx_fast(out=rc, in_=rc0)
                bc = bcp.tile([D, QB], f32, tag="bc", name="bc")
                nc.gpsimd.partition_broadcast(bc, rc, channels=D)
                if hh == 0:
                    nc.vector.tensor_mul(
                        aont[pc][0:D, Bc * QB : (Bc + 1) * QB], ao[hh][0:D, :], bc
                    )
                else:
                    # engine writes can't start at partition 64 from a base-0
                    # source; stage and partition-shift via DMA
                    tmp = tmpp.tile([D, QB], bf16, tag="tmp", name="tmp")
                    nc.vector.tensor_mul(tmp, ao[hh][0:D, :], bc)
                    nc.sync.dma_start(
                        out=aont[pc][D:P, Bc * QB : (Bc + 1) * QB], in_=tmp
                    )

            for gi in range(len(groups) + 1):
                prod = groups[gi] if gi < len(groups) else None
                cons = groups[gi - 1] if gi >= 1 else None
                if prod is not None:
                    pts[gi] = []
                if cons is not None:
                    ao = [aop.tile([D + 1, QB], f32, tag="ao", name="ao") for _ in range(2)]
                for kc in range(KC):
                    if proj_cooldown[0] > 0:
                        proj_cooldown[0] -= 1
                    else:
                        for _ in range(2 if prod is None else 1):
                            if proj_jobs:
                                proj_jobs.pop(0)()
                    if prod is not None:
                        B, p = prod
                        e = epp.tile([P, 2 * QB], f32, tag="ep", name="ep")
                        for hh in range(2):
                            i = 2 * p + hh
                            sl = slice(hh * D, (hh + 1) * D)
                            nc.tensor.matmul(
                                e[:, hh * QB : (hh + 1) * QB],
                                lhsT=kh[i][sl, kc * P : (kc + 1) * P],
                                rhs=qh[i][sl, B * QB : (B + 1) * QB],
                                start=True,
                                stop=True,
                            )
                        t = ptp.tile([P, 2 * QB], bf16, tag="pt", name="pt")
                        nc.scalar.activation(t, e, Exp, scale=SCALE)
                        pts[gi].append(t)
                    if cons is not None:
                        # h-major: h0's 16 AV matmuls over ticks 0-7, h1 over
                        # 8-15.  h1's ao-slot wait (previous group's h0
                        # normalize) hides behind h0's work, and each head's
                        # normalize chain starts half a group earlier.
                        Bc, pc = cons
                        hh = 0 if kc < KC // 2 else 1
                        for q in range(2):
                            k2 = (kc % (KC // 2)) * 2 + q
                            nc.tensor.matmul(
                                ao[hh],
                                lhsT=vt[2 * pc + hh][:, k2, :],
                                rhs=pts[gi - 1][k2][:, hh * QB : (hh + 1) * QB],
                                start=(k2 == 0),
                                stop=(k2 == KC - 1),
                            )
                        if kc == KC // 2 - 1:
                            normalize(cons, ao, 0)
                if cons is not None:
                    Bc, pc = cons
                    normalize(cons, ao, 1)
                    del pts[gi - 1]
                    if pc == 1:
                        # all 4 heads of qi block Bc are normalized: queue its
                        # projection, drip-fed into upcoming kc loops so it
                        # never blocks energy production (ScalarE supply).
                        # cooldown: don't pop the first job until the aont
                        # writes have had time to land (in-order PE queue).
                        emit_proj(Bc)
                        proj_cooldown[0] = 6
            for job in proj_jobs:
                job()

    nc.compile()
    return nc




def _program():
    global _PROGRAM
    if _PROGRAM is None:
        _PROGRAM = _build_program()
    return _PROGRAM


def _shard_inputs(values, keys, query, W_out):
    import ml_dtypes

    q = np.ascontiguousarray(np.asarray(query, np.float32)).reshape(N, S, H, D)
    k = np.ascontiguousarray(np.asarray(keys, np.float32)).reshape(N, S, H, D)
    v = np.ascontiguousarray(np.asarray(values, np.float32)).reshape(N, S, H, D)
    qT = np.ascontiguousarray(q.transpose(0, 2, 3, 1))  # [N, H, D, S]
    kT = np.ascontiguousarray(k.transpose(0, 2, 3, 1))
    vh = v.transpose(0, 2, 1, 3)  # [N, H, S, D] (view)
    WT = np.ascontiguousarray(np.asarray(W_out, np.float32).T)  # [E_in, E_out]

    in_maps = []
    for i in range(8):
        n, g = i // 4, i % 4
        h0 = 4 * g
        in_maps.append(
            {
                "qt": np.ascontiguousarray(qT[n, h0 : h0 + 4]).reshape(2, P, S).astype(ml_dtypes.bfloat16),
                "kt": np.ascontiguousarray(kT[n, h0 : h0 + 4]).reshape(2, P, S).astype(ml_dtypes.bfloat16),
                "v": np.ascontiguousarray(vh[n, h0 : h0 + 4]).astype(ml_dtypes.bfloat16),
                "wt": np.ascontiguousarray(WT[256 * g : 256 * (g + 1)]).reshape(2, P, E).astype(ml_dtypes.bfloat16),
            }
        )
    return in_maps


def kernel(values, keys, query, mask, W_out, b_out, _trace=False, _bkr_out=None):
    """Full inputs in, full output out.  mask is all-ones by construction and
    is ignored.  _trace/_bkr_out are test hooks (NTFF profiling)."""
    from concourse.bass_utils import run_bass_kernel_spmd

    nc = _program()
    in_maps = _shard_inputs(values, keys, query, W_out)
    bkr = run_bass_kernel_spmd(nc, in_maps, list(range(8)), trace=_trace)
    if _bkr_out is not None:
        _bkr_out.append(bkr)

    b = np.asarray(b_out, np.float32)
    out = np.empty((N, S, E), np.float32)
    for n in range(2):
        acc = bkr.results[4 * n]["out"].astype(np.float64)
        for j in range(1, 4):
            acc += bkr.results[4 * n + j]["out"]
        out[n] = (acc + b).astype(np.float32)
    return out

